# revision 1
# baseline (speedup 1.0000x reference)
"""Causal varlen self-attention (packed equal-length sequences) on 8 trn2 cores.

Sharding: 4 sequences x 2 head-groups. Core c handles sequence b = c//2 and
heads hh*8..hh*8+8 where hh = c%2. Each core computes QKV projection for its
sequence restricted to its heads, rotary+RMSNorm, causal attention for all
1024 rows of the sequence over its 8 heads, and a partial output projection
over its 512 features. The pair of cores for a sequence all-reduce their
partial y so every core ends with the full [1024, 1024] output of its
sequence; the host slices core 2b's output.

All matmuls run in float32r (TF32-like, ~1e-4 rel err, 3.4x faster than f32).
Softmax uses exp without max subtraction (RMS-normed q,k bound scores to
|s| <= 8) in a transposed scores layout [kpos, q], which avoids transposing
the probabilities for the PV matmul. Denominators come from a ones column
appended to V; per-head normalization happens on the small attention output.
"""
import numpy as np

N_EMBD = 1024
N_HEAD = 16
HD = 64
S = 1024
B = 4
N = B * S
NCORES = 8
HPC = 8           # heads per core
NHC = HPC // 2    # head-pair chunks per core
NB = S // 128     # row blocks per sequence
ND = N_EMBD // 128  # contraction chunks
JW = 3 * HPC * HD   # qkv feature width per core (1536)
NEG = -30000.0
RMS_EPS = 1.1920929e-07

_cached = {}


def _build():
    import concourse.bacc as bacc
    import concourse.mybir as mybir
    import concourse.tile as tile
    import concourse.bass as bass
    from concourse.masks import make_identity

    F32 = mybir.dt.float32
    F32R = mybir.dt.float32r

    nc = bacc.Bacc('TRN2', target_bir_lowering=False, debug=False,
                   num_devices=NCORES)
    xs = nc.dram_tensor('xs', [S, N_EMBD], F32, kind='ExternalInput').ap()
    wqkvT = nc.dram_tensor('wqkvT', [N_EMBD, JW], F32, kind='ExternalInput').ap()
    woT = nc.dram_tensor('woT', [HPC * HD, N_EMBD], F32, kind='ExternalInput').ap()
    cosg = nc.dram_tensor('cosg', [S, HD // 2], F32, kind='ExternalInput').ap()
    sing = nc.dram_tensor('sing', [S, HD // 2], F32, kind='ExternalInput').ap()
    ypart = nc.dram_tensor('ypart', [S, N_EMBD], F32, kind='ExternalOutput').ap()
    ystage = nc.dram_tensor('ystage', [S, N_EMBD], F32).ap()
    yred = nc.dram_tensor('yred', [S, N_EMBD], F32).ap()

    def bcast_mid(t, n, width):
        # view [128, width] tile as [128, n, width] broadcasting over middle dim
        return bass.AP(tensor=t.tensor, offset=t.offset,
                       ap=[t.ap[0], [0, n], t.ap[-1]])

    def bcast_last(t, width):
        # view [128, n] tile as [128, n, width] broadcasting over last dim
        return bass.AP(tensor=t.tensor, offset=t.offset,
                       ap=[t.ap[0], t.ap[1], [0, width]])

    with tile.TileContext(nc) as tc:
        import contextlib
        ctx = contextlib.ExitStack()
        with ctx:
            const = ctx.enter_context(tc.tile_pool(name='const', bufs=1))
            persist = ctx.enter_context(tc.tile_pool(name='persist', bufs=1))

            ident = const.tile([128, 128], F32)
            make_identity(nc, ident)
            # additive causal maskT[k, q] = 0 if k <= q else NEG
            maskT = const.tile([128, 128], F32)
            nc.gpsimd.memset(maskT, 0.0)
            nc.gpsimd.affine_select(
                out=maskT, in_=maskT, compare_op=mybir.AluOpType.is_ge,
                fill=NEG, base=0, pattern=[[1, 128]], channel_multiplier=-1)
            epst = const.tile([128, 1], F32)
            nc.vector.memset(epst, RMS_EPS)

            qT = [persist.tile([128, S], F32R, name=f'qT{i}') for i in range(NHC)]
            kT = [persist.tile([128, S], F32R, name=f'kT{i}') for i in range(NHC)]
            # v per head padded to 128 cols: cols 0:64 = v, 64:128 = ones, so the
            # PV matmul also produces 64 replicated denominator rows (free-dim
            # streaming cost is unchanged by M).
            vt = [persist.tile([128, HPC, 128], F32R, name=f'vt{i}') for i in range(NB)]
            attT = [persist.tile([128, S], F32R, name=f'attT{i}') for i in range(NHC)]

            # ---- phase 1+2: x transpose and QKV projection ----
            with tc.tile_pool(name='xtp', bufs=1) as xtp, \
                 tc.tile_pool(name='wqp', bufs=1) as wqp, \
                 tc.tile_pool(name='qkvwork', bufs=2) as qw, \
                 tc.tile_pool(name='scratch', bufs=2) as scratch, \
                 tc.tile_pool(name='pst', bufs=2, space='PSUM') as pst, \
                 tc.tile_pool(name='psq', bufs=2, space='PSUM') as psq:
                xT = [xtp.tile([128, S], F32R, name=f'xT{d}') for d in range(ND)]
                JH = JW // 2  # j-half width: [q | k0-3] then [k4-7 | v]

                for nb in range(NB):
                    xrow = qw.tile([128, N_EMBD], F32, tag='xrow')
                    nc.sync.dma_start(out=xrow, in_=xs[nb * 128:(nb + 1) * 128])
                    for d in range(ND):
                        pt = pst.tile([128, 128], F32, tag='pt')
                        nc.tensor.transpose(pt, xrow[:, d * 128:(d + 1) * 128], ident)
                        nc.vector.tensor_copy(xT[d][:, nb * 128:(nb + 1) * 128], pt)

                cost = [const.tile([128, HD // 2], F32, name=f'cos{i}') for i in range(NB)]
                sint = [const.tile([128, HD // 2], F32, name=f'sin{i}') for i in range(NB)]
                for nb in range(NB):
                    nc.sync.dma_start(out=cost[nb], in_=cosg[nb * 128:(nb + 1) * 128])
                    nc.sync.dma_start(out=sint[nb], in_=sing[nb * 128:(nb + 1) * 128])

                def rotary_rms(src, heads, is_q, cb, sb):
                    # src: [128, len(heads), 64] f32 view; returns normalized tile
                    nh = src.shape[1]
                    x1 = src[:, :, 0:32]
                    x2 = src[:, :, 32:64]
                    rot = scratch.tile([128, nh, HD], F32, tag='rot')
                    ra = scratch.tile([128, nh, 32], F32, tag='ra')
                    rb = scratch.tile([128, nh, 32], F32, tag='rb')
                    nc.vector.tensor_mul(ra, x1, cb)
                    nc.vector.tensor_mul(rb, x2, sb)
                    nc.vector.tensor_add(rot[:, :, 0:32], ra, rb)
                    nc.vector.tensor_mul(ra, x2, cb)
                    nc.vector.tensor_mul(rb, x1, sb)
                    nc.vector.tensor_tensor(out=rot[:, :, 32:64], in0=ra, in1=rb,
                                            op=mybir.AluOpType.subtract)
                    sq = scratch.tile([128, nh, HD], F32, tag='sq')
                    nc.vector.tensor_mul(sq, rot, rot)
                    ms = scratch.tile([128, nh], F32, tag='ms')
                    nc.vector.reduce_sum(out=ms, in_=sq, axis=mybir.AxisListType.X)
                    nc.scalar.activation(out=ms, in_=ms,
                                         func=mybir.ActivationFunctionType.Sqrt,
                                         bias=epst, scale=1.0 / HD)
                    nc.vector.reciprocal(out=ms, in_=ms)
                    if is_q:
                        nc.scalar.mul(out=ms, in_=ms, mul=HD ** -0.5)
                    nc.vector.tensor_mul(rot, rot, bcast_last(ms, HD))
                    return rot

                for jh in range(2):
                    wq = []
                    for d in range(ND):
                        wq32 = qw.tile([128, JH], F32, tag='wq32')
                        nc.sync.dma_start(
                            out=wq32, in_=wqkvT[d * 128:(d + 1) * 128, jh * JH:(jh + 1) * JH])
                        wqd = wqp.tile([128, JH], F32R, tag=f'wq{d}', name=f'wq{jh}_{d}')
                        nc.vector.tensor_copy(wqd, wq32)
                        wq.append(wqd)
                    for nb in range(NB):
                        pq = psq.tile([128, JH], F32, tag='pq')
                        for d in range(ND):
                            nc.tensor.matmul(
                                pq[:, 0:512],
                                xT[d][:, nb * 128:(nb + 1) * 128],
                                wq[d][:, 0:512],
                                start=(d == 0), stop=(d == ND - 1))
                            nc.tensor.matmul(
                                pq[:, 512:JH],
                                xT[d][:, nb * 128:(nb + 1) * 128],
                                wq[d][:, 512:JH],
                                start=(d == 0), stop=(d == ND - 1))
                        qkvs = qw.tile([128, JH // HD, HD], F32, tag='qkvs')
                        nc.vector.tensor_copy(qkvs, pq)

                        cb2 = bcast_mid(cost[nb], HPC, HD // 2)
                        sb2 = bcast_mid(sint[nb], HPC, HD // 2)
                        cb1 = bcast_mid(cost[nb], HPC // 2, HD // 2)
                        sb1 = bcast_mid(sint[nb], HPC // 2, HD // 2)
                        if jh == 0:
                            # q heads 0-7 then k heads 0-3
                            rotq = rotary_rms(qkvs[:, 0:HPC, :], HPC, True, cb2, sb2)
                            for hc in range(NHC):
                                pt2 = pst.tile([128, 128], F32, tag='pt')
                                nc.tensor.transpose(
                                    pt2, rotq[:, hc * 2:(hc + 1) * 2, :].rearrange("p a b -> p (a b)"),
                                    ident)
                                nc.vector.tensor_copy(qT[hc][:, nb * 128:(nb + 1) * 128], pt2)
                            rotk = rotary_rms(qkvs[:, HPC:HPC + 4, :], 4, False, cb1, sb1)
                            for hc in range(2):
                                pt2 = pst.tile([128, 128], F32, tag='pt')
                                nc.tensor.transpose(
                                    pt2, rotk[:, hc * 2:(hc + 1) * 2, :].rearrange("p a b -> p (a b)"),
                                    ident)
                                nc.vector.tensor_copy(kT[hc][:, nb * 128:(nb + 1) * 128], pt2)
                        else:
                            # k heads 4-7 then v heads 0-7
                            rotk = rotary_rms(qkvs[:, 0:4, :], 4, False, cb1, sb1)
                            for hc in range(2):
                                pt2 = pst.tile([128, 128], F32, tag='pt')
                                nc.tensor.transpose(
                                    pt2, rotk[:, hc * 2:(hc + 1) * 2, :].rearrange("p a b -> p (a b)"),
                                    ident)
                                nc.vector.tensor_copy(kT[2 + hc][:, nb * 128:(nb + 1) * 128], pt2)
                            nc.vector.tensor_copy(out=vt[nb][:, :, 0:HD], in_=qkvs[:, 4:4 + HPC, :])
                            nc.vector.memset(vt[nb][:, :, HD:128].bitcast(F32), 1.0)

            # ---- phase 3+4: attention interleaved with output projection ----
            with tc.tile_pool(name='estp', bufs=4) as estp, \
                 tc.tile_pool(name='attw', bufs=2) as attw, \
                 tc.tile_pool(name='wop', bufs=1) as wop, \
                 tc.tile_pool(name='ywork', bufs=3) as yw, \
                 tc.tile_pool(name='pssc', bufs=2, space='PSUM') as pssc, \
                 tc.tile_pool(name='pspv', bufs=1, space='PSUM') as pspv, \
                 tc.tile_pool(name='psy', bufs=2, space='PSUM') as psy:
                wo = [wop.tile([128, N_EMBD], F32R, name=f'wo{f}') for f in range(NHC)]
                for f in range(NHC):
                    wo32 = yw.tile([128, N_EMBD], F32, tag='wo32')
                    nc.sync.dma_start(out=wo32, in_=woT[f * 128:(f + 1) * 128])
                    nc.vector.tensor_copy(wo[f], wo32)
                for qg in range(2):
                    for hc in range(NHC):
                        nkc = 4 + qg * 4
                        pvs = [pspv.tile([128, 512], F32, name=f'pv{qg}_{hc}_{h2}', tag=f'pv{h2}')
                               for h2 in range(2)]
                        for kc in range(nkc):
                            vs = max(0, kc - qg * 4) * 128
                            diag = kc >= qg * 4
                            # both heads in one 2-bank psum tile
                            sct = pssc.tile([128, 2, 512], F32, tag='sc')
                            for h2 in range(2):
                                nc.tensor.matmul(
                                    sct[:, h2],
                                    kT[hc][h2 * HD:(h2 + 1) * HD, kc * 128:(kc + 1) * 128],
                                    qT[hc][h2 * HD:(h2 + 1) * HD, qg * 512:(qg + 1) * 512],
                                    start=True, stop=True,
                                    tile_position=(h2 * HD, 0))
                            if diag:
                                nc.vector.tensor_add(
                                    sct[:, :, vs:vs + 128], sct[:, :, vs:vs + 128],
                                    bcast_mid(maskT, 2, 128))
                            est = estp.tile([128, 2, 512], F32R, tag='est')
                            if vs > 0:
                                nc.vector.memset(est[:, :, 0:vs].bitcast(F32), 0.0)
                            nc.scalar.activation(out=est[:, :, vs:], in_=sct[:, :, vs:],
                                                 func=mybir.ActivationFunctionType.Exp)
                            for h2 in range(2):
                                nc.tensor.matmul(
                                    pvs[h2], vt[kc][:, hc * 2 + h2], est[:, h2],
                                    start=(kc == 0), stop=(kc == nkc - 1))
                        for h2 in range(2):
                            denr = attw.tile([HD, 512], F32, tag='denr')
                            nc.vector.reciprocal(denr, pvs[h2][HD:128, :])
                            nc.vector.tensor_mul(
                                attT[hc][h2 * HD:(h2 + 1) * HD, qg * 512:(qg + 1) * 512],
                                pvs[h2][0:HD, :], denr)

                    # project this half's rows while the other half's attention runs
                    for qt in range(qg * 4, qg * 4 + 4):
                        for og in range(2):
                            py = psy.tile([128, 512], F32, tag='py')
                            for f in range(NHC):
                                nc.tensor.matmul(
                                    py,
                                    attT[f][:, qt * 128:(qt + 1) * 128],
                                    wo[f][:, og * 512:(og + 1) * 512],
                                    start=(f == 0), stop=(f == NHC - 1))
                            ys = yw.tile([128, 512], F32, tag='ys')
                            nc.vector.tensor_copy(ys, py)
                            nc.sync.dma_start(
                                out=ystage[qt * 128:(qt + 1) * 128, og * 512:(og + 1) * 512],
                                in_=ys)
                    rs = slice(qg * 512, (qg + 1) * 512)
                    nc.gpsimd.collective_compute(
                        "AllReduce", mybir.AluOpType.add,
                        replica_groups=[[0, 1], [2, 3], [4, 5], [6, 7]],
                        ins=[ystage[rs, :]], outs=[yred[rs, :]])
                    nc.sync.dma_start(out=ypart[rs, :], in_=yred[rs, :])

    nc.compile()
    return nc


def _get_nc():
    if 'nc' not in _cached:
        _cached['nc'] = _build()
    return _cached['nc']


def kernel(x, Wqkv, Wo, cos_cache, sin_cache, cu_seqlens, position_ids,
           max_seqlen, **_ignored):
    from concourse.bass_utils import run_bass_kernel_spmd

    x = np.asarray(x)
    Wqkv = np.asarray(Wqkv)
    Wo = np.asarray(Wo)
    cos_cache = np.asarray(cos_cache)
    sin_cache = np.asarray(sin_cache)
    position_ids = np.asarray(position_ids)

    nc = _get_nc()
    in_maps = []
    for c in range(NCORES):
        b, hh = c // 2, c % 2
        rows = slice(b * S, (b + 1) * S)
        qsl = slice(hh * HPC * HD, (hh + 1) * HPC * HD)
        ksl = slice(N_EMBD + hh * HPC * HD, N_EMBD + (hh + 1) * HPC * HD)
        vsl = slice(2 * N_EMBD + hh * HPC * HD, 2 * N_EMBD + (hh + 1) * HPC * HD)
        wqkvT_c = np.concatenate(
            [Wqkv[qsl], Wqkv[ksl], Wqkv[vsl]], axis=0).T.copy()
        woT_c = Wo[:, qsl].T.copy()
        pos = position_ids[rows]
        in_maps.append({
            'xs': np.ascontiguousarray(x[rows]),
            'wqkvT': np.ascontiguousarray(wqkvT_c),
            'woT': np.ascontiguousarray(woT_c),
            'cosg': np.ascontiguousarray(cos_cache[pos]),
            'sing': np.ascontiguousarray(sin_cache[pos]),
        })

    r = run_bass_kernel_spmd(nc, in_maps, list(range(NCORES)))
    out = np.empty((N, N_EMBD), dtype=np.float32)
    for b in range(B):
        out[b * S:(b + 1) * S] = r.results[2 * b]['ypart']
    _cached['last_results'] = r
    return out



# revision 7
# speedup vs baseline: 1.8378x; 1.8378x over previous
"""Causal varlen self-attention (packed equal-length sequences) on 8 trn2 cores.

Sharding: 4 sequences x 2 head-groups. Core c handles sequence b = c//2 and
heads hh*8..hh*8+8 (hh = c%2). Each core computes the QKV projection of its
sequence restricted to its 8 heads, rotary+RMSNorm, causal attention for all
1024 rows over its heads, exchanges bf16 attention outputs with its pair
partner via AllGather, and computes the final output projection for its
512-wide column slice of y (even core: out cols 0..512, odd: 512..1024) over
the full 1024-feature contraction. The host assembles y column-wise -- the
program is SPMD-symmetric with no all-reduce.

All matmul inputs are bf16 (f32 PSUM accumulation); the host pre-transposes x
and pre-converts weights. Scores are computed in a transposed layout
[kpos, q] with causal column-trimming (matmuls/exp/PV only touch q >= kc*128);
within-diagonal-block masking zeroes est's upper triangle on the gpsimd
engine after exp. The k-side RMS norm is folded into exp's per-partition
scale; softmax denominators come from 64 ones-columns appended to V and one
vector divide per head normalizes the attention output.
"""
import numpy as np

N_EMBD = 1024
N_HEAD = 16
HD = 64
S = 1024
B = 4
N = B * S
NCORES = 8
HPC = 8            # heads per core
NHC = HPC // 2     # head-pair chunks per core
NB = S // 128      # row blocks per sequence
ND = N_EMBD // 128  # contraction chunks
JW = 3 * HPC * HD  # qkv feature width per core (1536)
OW = N_EMBD // 2   # output columns per core (512)
RMS_EPS = 1.1920929e-07

_cached = {}


def _build():
    import concourse.bacc as bacc
    import concourse.mybir as mybir
    import concourse.tile as tile
    import concourse.bass as bass
    from concourse.masks import make_identity

    F32 = mybir.dt.float32
    BF16 = mybir.dt.bfloat16
    ALU = mybir.AluOpType
    ACT = mybir.ActivationFunctionType

    nc = bacc.Bacc('TRN2', target_bir_lowering=False, debug=False,
                   num_devices=NCORES)
    xT = nc.dram_tensor('xT', [N_EMBD, S], BF16, kind='ExternalInput').ap()
    wqkvT = nc.dram_tensor('wqkvT', [N_EMBD, JW], BF16, kind='ExternalInput').ap()
    woT = nc.dram_tensor('woT', [N_EMBD, OW], BF16, kind='ExternalInput').ap()
    cosg = nc.dram_tensor('cosg', [S, HD // 2], BF16, kind='ExternalInput').ap()
    sing = nc.dram_tensor('sing', [S, HD // 2], BF16, kind='ExternalInput').ap()
    yhalf = nc.dram_tensor('yhalf', [S, OW], F32, kind='ExternalOutput').ap()
    attstage = nc.dram_tensor('attstage', [NHC, 128, S], BF16).ap()
    # two AllGathers, each over 2 local head-pairs: out [rank, hp, 128, S]
    ag = [nc.dram_tensor(f'ag{g}', [2, 2, 128, S], BF16).ap() for g in range(2)]

    def bcast_mid(t, n):
        # view [128, w] tile as [128, n, w] broadcasting over middle dim
        return bass.AP(tensor=t.tensor, offset=t.offset,
                       ap=[t.ap[0], [0, n], t.ap[-1]])

    def bcast_last(t, width):
        # view [128, n] tile as [128, n, width] broadcasting over last dim
        return bass.AP(tensor=t.tensor, offset=t.offset,
                       ap=[t.ap[0], t.ap[1], [0, width]])

    with tile.TileContext(nc) as tc:
        import contextlib
        ctx = contextlib.ExitStack()
        with ctx:
            const = ctx.enter_context(tc.tile_pool(name='const', bufs=1))
            persist = ctx.enter_context(tc.tile_pool(name='persist', bufs=1))

            ident = const.tile([128, 128], BF16)
            make_identity(nc, ident)
            epst = const.tile([128, 1], F32)
            nc.vector.memset(epst, RMS_EPS)

            # persistent SBUF data
            xTs = [persist.tile([128, S], BF16, name=f'xTs{d}') for d in range(ND)]
            wq = [persist.tile([128, JW], BF16, name=f'wq{d}') for d in range(ND)]
            wo = [persist.tile([128, OW], BF16, name=f'wo{f}') for f in range(2 * NHC)]
            cosb = [const.tile([128, HD // 2], BF16, name=f'cos{i}') for i in range(NB)]
            sinb = [const.tile([128, HD // 2], BF16, name=f'sin{i}') for i in range(NB)]
            qT = [persist.tile([128, S], BF16, name=f'qT{i}') for i in range(NHC)]
            kT = [persist.tile([128, S], BF16, name=f'kT{i}') for i in range(NHC)]
            vt = [persist.tile([128, HPC, 128], BF16, name=f'vt{i}') for i in range(NB)]
            attT = [persist.tile([128, S], BF16, name=f'attT{f}') for f in range(NHC)]
            attG = [persist.tile([128, S], BF16, name=f'attG{f}') for f in range(2 * NHC)]
            # rnkt[nb][:, 0:8] = q-norm recip (with HD^-0.5), [:, 8:16] = k-norm recip
            rnkt = [persist.tile([128, N_HEAD], F32, name=f'rn{i}') for i in range(NB)]

            # prologue DMAs, interleaved so block-0 work can start early
            for d in range(ND):
                nc.sync.dma_start(out=xTs[d], in_=xT[d * 128:(d + 1) * 128])
                nc.sync.dma_start(out=wq[d], in_=wqkvT[d * 128:(d + 1) * 128])
            for nb in range(NB):
                nc.sync.dma_start(out=cosb[nb], in_=cosg[nb * 128:(nb + 1) * 128])
                nc.sync.dma_start(out=sinb[nb], in_=sing[nb * 128:(nb + 1) * 128])
            for f in range(2 * NHC):
                nc.sync.dma_start(out=wo[f], in_=woT[f * 128:(f + 1) * 128])
            for nb in range(NB):
                nc.gpsimd.memset(vt[nb][:, :, HD:128], 1.0)

            # ---- phase 1: QKV projection + rotary + rms + transposes ----
            with tc.tile_pool(name='qkw', bufs=2) as qkw, \
                 tc.tile_pool(name='rotw', bufs=2) as rotw, \
                 tc.tile_pool(name='psq', bufs=2, space='PSUM') as psq, \
                 tc.tile_pool(name='ptr', bufs=2, space='PSUM') as ptr:
                for nb in range(NB):
                    rsl = slice(nb * 128, (nb + 1) * 128)
                    pq = psq.tile([128, 3 * HPC, HD], F32, tag='pq')
                    for d in range(ND):
                        for g in range(3):
                            nc.tensor.matmul(
                                pq[:, g * HPC:(g + 1) * HPC],
                                xTs[d][:, rsl],
                                wq[d][:, g * 512:(g + 1) * 512],
                                start=(d == 0), stop=(d == ND - 1))
                    # evacuate psum: q,k -> bf16 for rotary; v -> vt
                    qk = qkw.tile([128, N_HEAD, HD], BF16, tag='qk')
                    nc.scalar.copy(qk, pq[:, 0:N_HEAD])
                    nc.scalar.copy(vt[nb][:, :, 0:HD], pq[:, N_HEAD:3 * HPC])

                    # rotary on q+k heads together (all bf16, 2x DVE)
                    cb = bcast_mid(cosb[nb], N_HEAD)
                    sb = bcast_mid(sinb[nb], N_HEAD)
                    x1 = qk[:, :, 0:32]
                    x2 = qk[:, :, 32:64]
                    rot = rotw.tile([128, N_HEAD, HD], BF16, tag='rot')
                    scr = rotw.tile([128, N_HEAD, HD], BF16, tag='scr')
                    nc.vector.tensor_tensor(out=rot[:, :, 0:32], in0=x1, in1=cb, op=ALU.mult)
                    nc.vector.tensor_tensor(out=scr[:, :, 0:32], in0=x2, in1=sb, op=ALU.mult)
                    nc.vector.tensor_tensor(out=rot[:, :, 0:32], in0=rot[:, :, 0:32],
                                            in1=scr[:, :, 0:32], op=ALU.add)
                    nc.vector.tensor_tensor(out=rot[:, :, 32:64], in0=x2, in1=cb, op=ALU.mult)
                    nc.vector.tensor_tensor(out=scr[:, :, 32:64], in0=x1, in1=sb, op=ALU.mult)
                    nc.vector.tensor_tensor(out=rot[:, :, 32:64], in0=rot[:, :, 32:64],
                                            in1=scr[:, :, 32:64], op=ALU.subtract)

                    # rms: ms = sum(rot^2) over head dim; rn = 1/sqrt(ms/64+eps)
                    nc.vector.tensor_tensor(out=scr, in0=rot, in1=rot, op=ALU.mult)
                    ms = qkw.tile([128, N_HEAD], F32, tag='ms')
                    nc.vector.reduce_sum(out=ms, in_=scr, axis=mybir.AxisListType.X)
                    nc.scalar.activation(out=ms, in_=ms, func=ACT.Sqrt,
                                         bias=epst, scale=1.0 / HD)
                    nc.vector.reciprocal(out=rnkt[nb], in_=ms)
                    # fold HD^-0.5 into the q-side recips (bf16 for the mul)
                    rnq = qkw.tile([128, HPC], BF16, tag='rnq')
                    nc.scalar.mul(out=rnq, in_=rnkt[nb][:, 0:HPC], mul=HD ** -0.5)
                    nc.vector.tensor_tensor(out=rot[:, 0:HPC, :], in0=rot[:, 0:HPC, :],
                                            in1=bcast_last(rnq, HD), op=ALU.mult)

                    # transpose q (normalized) and k (unnormalized) head-pairs
                    for hc in range(NHC):
                        pt = ptr.tile([128, 128], BF16, tag='pt')
                        nc.tensor.transpose(
                            pt, rot[:, 2 * hc:2 * hc + 2, :].rearrange("p a b -> p (a b)"),
                            ident)
                        nc.vector.tensor_copy(qT[hc][:, rsl], pt)
                        pt2 = ptr.tile([128, 128], BF16, tag='pt')
                        nc.tensor.transpose(
                            pt2, rot[:, HPC + 2 * hc:HPC + 2 * hc + 2, :].rearrange("p a b -> p (a b)"),
                            ident)
                        nc.scalar.copy(kT[hc][:, rsl], pt2)

            # ---- phase 2: attention (scores^T -> exp -> mask -> PV) ----
            with tc.tile_pool(name='estp', bufs=3) as estp, \
                 tc.tile_pool(name='pssc', bufs=2, space='PSUM') as pssc, \
                 tc.tile_pool(name='pspv', bufs=2, space='PSUM') as pspv:
                for h in range(HPC):
                    hc, h2 = h // 2, h % 2
                    dsl = slice(h2 * HD, (h2 + 1) * HD)
                    pv = pspv.tile([128, S], F32, tag='pv')
                    for kc in range(NB):
                        c0 = kc * 128
                        chunks = [(c0, 512), (512, S)] if c0 < 512 else [(c0, S)]
                        sct = pssc.tile([128, S], F32, tag='sct')
                        for a, b in chunks:
                            nc.tensor.matmul(
                                sct[:, a:b],
                                kT[hc][dsl, c0:c0 + 128],
                                qT[hc][dsl, a:b],
                                start=True, stop=True)
                        est = estp.tile([128, S], BF16, tag='est')
                        nc.scalar.activation(out=est[:, c0:], in_=sct[:, c0:],
                                             func=ACT.Exp,
                                             scale=rnkt[kc][:, HPC + h:HPC + h + 1])
                        # zero the strictly-upper triangle of the diagonal block
                        nc.gpsimd.affine_select(
                            out=est[:, c0:c0 + 128], in_=est[:, c0:c0 + 128],
                            compare_op=ALU.is_ge, fill=0.0, base=0,
                            pattern=[[1, 128]], channel_multiplier=-1)
                        for a, b in chunks:
                            nc.tensor.matmul(
                                pv[:, a:b], vt[kc][:, h], est[:, a:b],
                                start=(kc == 0), stop=(kc == NB - 1),
                                skip_group_check=True)
                    # normalize by the ones-column denominators (rows 64:128)
                    rden = estp.tile([HD, S], F32, tag='rden')
                    nc.vector.reciprocal(out=rden, in_=pv[HD:128, :])
                    nc.vector.tensor_tensor(out=attT[hc][dsl, :], in0=pv[0:HD, :],
                                            in1=rden, op=ALU.mult)
                    if h2 == 1:
                        nc.sync.dma_start(out=attstage[hc], in_=attT[hc])
                    if h == 3:
                        nc.gpsimd.collective_compute(
                            "AllGather", ALU.bypass,
                            replica_groups=[[0, 1], [2, 3], [4, 5], [6, 7]],
                            ins=[attstage[0:2]], outs=[ag[0]])
                    if h == 7:
                        nc.gpsimd.collective_compute(
                            "AllGather", ALU.bypass,
                            replica_groups=[[0, 1], [2, 3], [4, 5], [6, 7]],
                            ins=[attstage[2:4]], outs=[ag[1]])
                # readback: attG[global head-pair] for both ranks' contributions
                for g in range(2):
                    for r in range(2):
                        for j in range(2):
                            gf = r * NHC + g * 2 + j
                            nc.sync.dma_start(out=attG[gf], in_=ag[g][r, j])

            # ---- phase 3: output projection over all 16 heads ----
            with tc.tile_pool(name='yw', bufs=3) as yw, \
                 tc.tile_pool(name='psy', bufs=4, space='PSUM') as psy:
                for qt in range(NB):
                    py = psy.tile([128, OW], F32, tag='py')
                    for gf in range(2 * NHC):
                        nc.tensor.matmul(
                            py, attG[gf][:, qt * 128:(qt + 1) * 128], wo[gf],
                            start=(gf == 0), stop=(gf == 2 * NHC - 1))
                    ys = yw.tile([128, OW], F32, tag='ys')
                    if qt % 2 == 0:
                        nc.vector.tensor_copy(ys, py)
                    else:
                        nc.scalar.copy(ys, py)
                    nc.sync.dma_start(out=yhalf[qt * 128:(qt + 1) * 128], in_=ys)

    nc.compile()
    return nc


def _get_nc():
    if 'nc' not in _cached:
        _cached['nc'] = _build()
    return _cached['nc']


def kernel(x, Wqkv, Wo, cos_cache, sin_cache, cu_seqlens, position_ids,
           max_seqlen, **_ignored):
    from concourse.bass_utils import run_bass_kernel_spmd
    import ml_dtypes

    bf16 = ml_dtypes.bfloat16
    x = np.asarray(x, dtype=np.float32)
    Wqkv = np.asarray(Wqkv, dtype=np.float32)
    Wo = np.asarray(Wo, dtype=np.float32)
    cos_cache = np.asarray(cos_cache, dtype=np.float32)
    sin_cache = np.asarray(sin_cache, dtype=np.float32)
    position_ids = np.asarray(position_ids)

    nc = _get_nc()
    in_maps = []
    for c in range(NCORES):
        b, hh = c // 2, c % 2
        rows = slice(b * S, (b + 1) * S)
        qsl = slice(hh * HPC * HD, (hh + 1) * HPC * HD)
        ksl = slice(N_EMBD + hh * HPC * HD, N_EMBD + (hh + 1) * HPC * HD)
        vsl = slice(2 * N_EMBD + hh * HPC * HD, 2 * N_EMBD + (hh + 1) * HPC * HD)
        wqkvT_c = np.concatenate(
            [Wqkv[qsl], Wqkv[ksl], Wqkv[vsl]], axis=0).T
        woT_c = Wo[hh * OW:(hh + 1) * OW, :].T
        pos = position_ids[rows]
        in_maps.append({
            'xT': np.ascontiguousarray(x[rows].T).astype(bf16),
            'wqkvT': np.ascontiguousarray(wqkvT_c).astype(bf16),
            'woT': np.ascontiguousarray(woT_c).astype(bf16),
            'cosg': np.ascontiguousarray(cos_cache[pos]).astype(bf16),
            'sing': np.ascontiguousarray(sin_cache[pos]).astype(bf16),
        })

    r = run_bass_kernel_spmd(nc, in_maps, list(range(NCORES)))
    out = np.empty((N, N_EMBD), dtype=np.float32)
    for b in range(B):
        rows = slice(b * S, (b + 1) * S)
        out[rows, 0:OW] = r.results[2 * b]['yhalf']
        out[rows, OW:N_EMBD] = r.results[2 * b + 1]['yhalf']
    _cached['last_results'] = r
    return out
